# revision 1
# baseline (speedup 1.0000x reference)
"""GATv2 self-attention kernel for 8 Trainium2 NeuronCores.

Sharding: one attention head per core (8 heads / 8 cores). Each core computes
its head's attn-weighted projection as a partial sum over heads, the cores
ReduceScatter the partials over the feature axis, and each core finishes its
256-column feature slice (bias-mean + residual) and returns it; the host
concatenates the 8 slices.

Math per head h (reference):
  X = inputs.reshape(B*S, F); x0 = rows of X with s == 0
  Wh = leaky_relu(X @ W2h + broadcast_s(x0 @ W1h))      [B*S, F]
  e  = Wh @ att_w[h]; attn = softmax_s(e)
  out = sum_h (attn * Wh)/H + mean_h(bias) + X

All matmuls run in float32r (TF32-like, 11-bit mantissa) on the PE array.
X is transposed on-chip via PE transpose-mode matmuls. The broadcast x0@W1
term is accumulated into the same PSUM group as the X@W2 tiles through a
0/1 selector matmul, so no elementwise broadcast-add pass is needed.
"""

import sys
import os
import numpy as np

sys.path.insert(0, "/opt/trn_rl_repo")

B, S, F, H = 256, 8, 2048, 8
BS = B * S            # 2048
NB = 1024             # bs-chunk size (2 chunks)
NCHUNK = BS // NB     # 2
FSLICE = F // H       # 256 output feature columns per core
ALPHA = 0.3
USE_PRELU = True

_cache = {}


def _build(reps=1):
    import concourse.bacc as bacc
    import concourse.mybir as mybir
    import concourse.tile as tile
    import concourse.bass as bass
    from concourse.masks import make_identity

    f32 = mybir.dt.float32
    f32r = mybir.dt.float32r
    AF = mybir.ActivationFunctionType
    OP = mybir.AluOpType

    nc = bacc.Bacc(num_devices=H)

    w1_in = nc.declare_dram_parameter(
        "w1t", [F // 256, 128, 2, F // 128, 128], f32, isOutput=False)
    w2_in = nc.declare_dram_parameter(
        "w2t", [F // 256, 128, 2, F // 128, 128], f32, isOutput=False)
    x_in = nc.declare_dram_parameter("x", [BS, F], f32, isOutput=False)
    attw_in = nc.declare_dram_parameter("attw", [F], f32, isOutput=False)
    sel_in = nc.declare_dram_parameter("sel", [128, NB], f32, isOutput=False)
    xres_in = nc.declare_dram_parameter("xres", [BS, FSLICE], f32, isOutput=False)
    bm_in = nc.declare_dram_parameter("bm", [FSLICE], f32, isOutput=False)
    out_ext = nc.declare_dram_parameter("out", [BS, FSLICE], f32, isOutput=True)

    from contextlib import ExitStack
    with tile.TileContext(nc) as tc:
        with ExitStack() as ctx:
            consts = ctx.enter_context(tc.tile_pool(name="consts", bufs=1))
            xtp = ctx.enter_context(tc.tile_pool(name="xtp", bufs=1))
            whp = ctx.enter_context(tc.tile_pool(name="whp", bufs=1))
            wblkp = ctx.enter_context(tc.tile_pool(name="wblk", bufs=1))
            wrp = ctx.enter_context(tc.tile_pool(name="wrp", bufs=2))
            x0cp = ctx.enter_context(tc.tile_pool(name="x0c", bufs=1))
            xnatp = ctx.enter_context(tc.tile_pool(name="xnat", bufs=2))
            t03p = ctx.enter_context(tc.tile_pool(name="t03", bufs=2))
            esmp = ctx.enter_context(tc.tile_pool(name="esm", bufs=1))
            abrsbp = ctx.enter_context(tc.tile_pool(name="abrsb", bufs=1))
            xrsp = ctx.enter_context(tc.tile_pool(name="xrs", bufs=1))
            outstp = ctx.enter_context(tc.tile_pool(name="outst", bufs=1))
            ypool = ctx.enter_context(tc.tile_pool(name="ypool", bufs=4, space="PSUM"))
            epool = ctx.enter_context(tc.tile_pool(name="epool", bufs=2, space="PSUM"))
            tpool = ctx.enter_context(tc.tile_pool(name="tpool", bufs=2, space="PSUM"))
            dpool = ctx.enter_context(tc.tile_pool(name="dram", bufs=2, space="DRAM"))

            # ---------------- constants ----------------
            ident = consts.tile([128, 128], f32)
            make_identity(nc, ident)

            attw_f = consts.tile([128, F // 128], f32)
            nc.sync.dma_start(out=attw_f, in_=attw_in.rearrange("(o p) -> p o", p=128))
            attw_r = consts.tile([128, F // 128], f32r)
            nc.vector.tensor_copy(attw_r, attw_f)

            sel_r = consts.tile([128, NB], f32r)
            sel_stg = xnatp.tile([128, F], f32, tag="xnat")
            nc.sync.dma_start(out=sel_stg[:, :NB], in_=sel_in[:, :])
            nc.vector.tensor_copy(sel_r, sel_stg[:, :NB])

            al_sb = consts.tile([128, 1], f32)
            nc.vector.memset(al_sb, ALPHA)

            bm_sb = consts.tile([128, FSLICE // 128], f32)
            nc.sync.dma_start(out=bm_sb, in_=bm_in.rearrange("(o p) -> p o", p=128))

            # view of x grouped by (b, s)
            x_bsf = x_in.rearrange("(b s) f -> b s f", s=S)

            for _rep in range(reps):
                _run_body(nc, tc, mybir, bass, f32, f32r, AF, OP,
                          make_identity, ident, attw_r, sel_r, bm_sb, al_sb, x_bsf,
                          w1_in, w2_in, x_in, xres_in, out_ext,
                          consts, xtp, whp, wblkp, wrp, x0cp, xnatp, t03p,
                          esmp, abrsbp, xrsp, outstp,
                          ypool, epool, tpool, dpool, _rep)

    nc.compile()
    return nc


def _run_body(nc, tc, mybir, bass, f32, f32r, AF, OP,
              make_identity, ident, attw_r, sel_r, bm_sb, al_sb, x_bsf,
              w1_in, w2_in, x_in, xres_in, out_ext,
              consts, xtp, whp, wblkp, wrp, x0cp, xnatp, t03p,
              esmp, abrsbp, xrsp, outstp,
              ypool, epool, tpool, dpool, rep):
    NFB = F // 128   # 16 fo/fi blocks
    # ---------------- prologue: X0 = x0 @ W1 ----------------
    # x0t: [128 fi, 16 fi_outer, 256 b] carved out of the Wh buffer slot
    x0t_full = whp.tile([128, NFB, NB], f32r, tag="wh")
    x0t = x0t_full[:, :, :B]
    for bt in range(B // 128):
        x0nat = xnatp.tile([128, F], f32, tag="xnat")
        nc.sync.dma_start(
            out=x0nat, in_=x_bsf[bt * 128:(bt + 1) * 128, 0, :])
        for fi in range(NFB):
            pt = tpool.tile([128, 128], f32, tag="tp")
            nc.tensor.transpose(
                pt, x0nat[:, fi * 128:(fi + 1) * 128], ident)
            nc.any.tensor_copy(
                out=x0t[:, fi, bt * 128:(bt + 1) * 128], in_=pt)

    # X0 in transposed layout first: X0_T [128 fo, 16 fo_outer, 256 b]
    x0T = x0t_full[:, :, B:2 * B]
    for pair in range(NFB // 2):
        wblk = wblkp.tile([128, 2, NFB, 128], f32, tag="wblk")
        nc.sync.dma_start(out=wblk, in_=w1_in[pair])
        for half in range(2):
            fb = pair * 2 + half
            wr = wrp.tile([128, NFB, 128], f32r, tag="wr")
            nc.vector.tensor_copy(wr, wblk[:, half])
            ps_full = ypool.tile([128, 512], f32, tag="yp")
            ps = ps_full[:, :B]
            for fi in range(NFB):
                nc.tensor.matmul(
                    ps, wr[:, fi, :], x0t[:, fi, :],
                    start=(fi == 0), stop=(fi == NFB - 1))
            nc.any.tensor_copy(out=x0T[:, fb, :], in_=ps)

    # transpose X0_T back to natural layout, stage to DRAM (fp32)
    x0_dram = dpool.tile([B // 128, 128, F], f32, tag="x0dram")
    for bt in range(B // 128):
        x0stg = xnatp.tile([128, F], f32, tag="xnat")
        for fb in range(NFB):
            pt = tpool.tile([128, 128], f32, tag="tp")
            nc.tensor.transpose(
                pt, x0T[:, fb, bt * 128:(bt + 1) * 128].bitcast(f32), ident)
            nc.any.tensor_copy(
                out=x0stg[:, fb * 128:(fb + 1) * 128], in_=pt)
        nc.sync.dma_start(out=x0_dram[bt], in_=x0stg)

    # ---------------- main loop over bs-chunks ----------------
    rs_list = []
    for c in range(NCHUNK):
        # -- build x_t chunk [128 fi, 16 fi_outer, NB bs] (f32r) --
        x_t = xtp.tile([128, NFB, NB], f32r, tag="xt")
        for bsub in range(NB // 128):
            r0 = c * NB + bsub * 128
            xnat = xnatp.tile([128, F], f32, tag="xnat")
            eng = nc.sync if bsub % 2 == 0 else nc.gpsimd
            eng.dma_start(out=xnat, in_=x_in[r0:r0 + 128, :])
            for fi in range(NFB):
                pt = tpool.tile([128, 128], f32, tag="tp")
                nc.tensor.transpose(
                    pt, xnat[:, fi * 128:(fi + 1) * 128], ident)
                nc.any.tensor_copy(
                    out=x_t[:, fi, bsub * 128:(bsub + 1) * 128], in_=pt)

        # X0 rows for this chunk: load + round to f32r
        x0ld = xnatp.tile([128, F], f32, tag="xnat")
        nc.sync.dma_start(out=x0ld, in_=x0_dram[c])
        x0c_r = x0cp.tile([128, F], f32r, tag="x0c", name=f"x0c{rep}_{c}")
        nc.vector.tensor_copy(x0c_r, x0ld)

        # -- Wh chunk [128 fo, 16 fo_outer, NB bs] --
        wh = whp.tile([128, NFB, NB], f32r, tag="wh")
        e_ps = []
        for _b2 in range(NB // 512):
            e_ps_t = epool.tile([1, 512], f32, tag="ep",
                                name=f"eps{rep}_{c}_{_b2}")
            e_ps.append(e_ps_t)

        pending_e = []  # lag e-matvecs one fo-block so PE never waits on ACT
        for pair in range(NFB // 2):
            wblk = wblkp.tile([128, 2, NFB, 128], f32, tag="wblk")
            nc.sync.dma_start(out=wblk, in_=w2_in[pair])
            for half in range(2):
                fo = pair * 2 + half
                wr = wrp.tile([128, NFB, 128], f32r, tag="wr")
                nc.vector.tensor_copy(wr, wblk[:, half])
                for b2 in range(NB // 512):
                    ps = ypool.tile([128, 512], f32, tag="yp")
                    for fi in range(NFB):
                        nc.tensor.matmul(
                            ps, wr[:, fi, :],
                            x_t[:, fi, b2 * 512:(b2 + 1) * 512],
                            start=(fi == 0), stop=False)
                    # + broadcast_s(X0): selector matmul closes the group
                    nc.tensor.matmul(
                        ps, x0c_r[:, fo * 128:(fo + 1) * 128],
                        sel_r[:, b2 * 512:(b2 + 1) * 512],
                        start=False, stop=True)
                    # leaky_relu on ACT (exact Prelu), writes f32r Wh
                    whs = wh[:, fo, b2 * 512:(b2 + 1) * 512]
                    if USE_PRELU:
                        nc.scalar.activation(whs, ps, AF.Prelu,
                                             alpha=al_sb[:, :])
                    else:
                        t03 = t03p.tile([128, 512], f32, tag="t03")
                        nc.scalar.activation(t03, ps, AF.Copy, scale=ALPHA)
                        nc.vector.tensor_tensor(
                            out=whs, in0=ps, in1=t03, op=OP.max)
                    pending_e.append((fo, b2, whs))
                # emit lagged e-matvecs (previous fo block)
                while len(pending_e) > NB // 512:
                    efo, eb2, ewhs = pending_e.pop(0)
                    nc.tensor.matmul(
                        e_ps[eb2], attw_r[:, efo:efo + 1], ewhs,
                        start=(efo == 0), stop=(efo == NFB - 1))
        for efo, eb2, ewhs in pending_e:
            nc.tensor.matmul(
                e_ps[eb2], attw_r[:, efo:efo + 1], ewhs,
                start=(efo == 0), stop=(efo == NFB - 1))

        # -- softmax over s (groups of 8 along bs), scaled by 1/H --
        ab_full = abrsbp.tile([128, FSLICE // 128, NB], f32,
                              tag="abrsb", name=f"abf{rep}_{c}")
        ab_sb = ab_full[:, 0, :]
        NG = 512 // S
        for b2 in range(NB // 512):
            e_sb = esmp.tile([1, 512], f32, tag="esb",
                             name=f"esb{rep}_{c}_{b2}")
            nc.scalar.activation(e_sb, e_ps[b2], AF.Copy)
            work = esmp.tile([1, 512], f32, tag="work",
                             name=f"work{rep}_{c}_{b2}")
            e3 = e_sb.rearrange("p (b s) -> p b s", s=S)
            w3 = work.rearrange("p (b s) -> p b s", s=S)
            mx = esmp.tile([1, NG], f32, tag="mx", name=f"mx{rep}_{c}_{b2}")
            nc.vector.reduce_max(out=mx, in_=e3, axis=mybir.AxisListType.X)
            nc.vector.tensor_tensor(
                out=w3, in0=e3, in1=mx[:, :, None].to_broadcast((1, NG, S)),
                op=OP.subtract)
            nc.scalar.activation(e_sb, work, AF.Exp)
            sm = esmp.tile([1, NG], f32, tag="sm", name=f"sm{rep}_{c}_{b2}")
            nc.vector.reduce_sum(out=sm, in_=e3, axis=mybir.AxisListType.X)
            rc = esmp.tile([1, NG], f32, tag="rc", name=f"rc{rep}_{c}_{b2}")
            nc.vector.reciprocal(rc, sm)
            nc.vector.tensor_scalar_mul(rc, rc, 1.0 / H)
            attn_sb = work
            nc.vector.tensor_tensor(
                out=w3, in0=e3, in1=rc[:, :, None].to_broadcast((1, NG, S)),
                op=OP.mult)
            attn_dram = dpool.tile([1, 512], f32, tag="attn_dram")
            nc.gpsimd.dma_start(out=attn_dram[:, :], in_=attn_sb)
            attn_bc = bass.AP(
                tensor=attn_dram.tensor,
                offset=attn_dram.offset,
                ap=[[0, 128]] + [list(p) for p in attn_dram[:, :].ap[1:]],
            )
            nc.gpsimd.dma_start(
                out=ab_sb[:, b2 * 512:(b2 + 1) * 512], in_=attn_bc)

        # -- partial = attn/H * Wh, in place, then 2 bulk DMAs --
        partial_c = dpool.tile([F, NB], f32, tag="partial")
        pview = partial_c.rearrange("(o p) n -> p o n", p=128)
        for fo in range(NFB):
            whs = wh[:, fo, :]
            nc.vector.tensor_tensor(
                out=whs, in0=whs.bitcast(f32), in1=ab_sb, op=OP.mult)
        for hh in range(2):
            nc.gpsimd.dma_start(
                out=pview[:, hh * 8:(hh + 1) * 8, :],
                in_=wh[:, hh * 8:(hh + 1) * 8, :].bitcast(f32))

        # -- reduce over heads: ReduceScatter along f-axis --
        rs_c = dpool.tile([FSLICE, NB], f32, tag="rs", name=f"rs{rep}_{c}")
        nc.gpsimd.collective_compute(
            "ReduceScatter", OP.add,
            replica_groups=[list(range(H))],
            ins=[partial_c[:, :]], outs=[rs_c[:, :]])
        rs_list.append(rs_c)

    # finish phase after all chunks (overlaps trailing collectives)
    for c in range(NCHUNK):
        _finish_chunk(nc, tc, mybir, bass, f32, f32r, AF, OP, ident, bm_sb,
                      xres_in, out_ext, abrsbp, xrsp, outstp, tpool,
                      rs_list[c], c, rep)


def _get_nc():
    if "nc" not in _cache:
        _cache["nc"] = _build()
    return _cache["nc"]


def _make_in_maps(inputs_dict):
    x = np.ascontiguousarray(
        np.asarray(inputs_dict["inputs"], dtype=np.float32).reshape(BS, F))
    W = np.asarray(inputs_dict["W"], dtype=np.float32)
    att_w = np.asarray(inputs_dict["att_w"], dtype=np.float32)
    bias = np.asarray(inputs_dict["bias"], dtype=np.float32)

    sel = np.repeat(np.eye(128, dtype=np.float32), S, axis=1)  # [128, 1024]
    bm_full = bias.mean(axis=0)  # [F]

    def tile_w(w):
        # [F, F] -> [pair, kp, b, ko, n]; fi = ko*128+kp, fo = pair*256+b*128+n
        t = w.reshape(F // 128, 128, F // 256, 2, 128)  # [ko, kp, pair, b, n]
        return np.ascontiguousarray(t.transpose(2, 1, 3, 0, 4))

    in_maps = []
    for i in range(H):
        in_maps.append({
            "w1t": tile_w(W[i, :F, :]),
            "w2t": tile_w(W[i, F:, :]),
            "x": x,
            "attw": np.ascontiguousarray(att_w[i]),
            "sel": sel,
            "xres": np.ascontiguousarray(x[:, FSLICE * i:FSLICE * (i + 1)]),
            "bm": np.ascontiguousarray(bm_full[FSLICE * i:FSLICE * (i + 1)]),
        })
    return in_maps


def kernel(inputs, W, att_w, bias):
    from concourse.bass_utils import run_bass_kernel_spmd

    nc = _get_nc()
    in_maps = _make_in_maps(
        {"inputs": inputs, "W": W, "att_w": att_w, "bias": bias})
    res = run_bass_kernel_spmd(nc, in_maps, list(range(H)))
    _cache["last_result"] = res

    out = np.concatenate([res.results[i]["out"] for i in range(H)], axis=1)
    return out.reshape(B, S, F)


def _finish_chunk(nc, tc, mybir, bass, f32, f32r, AF, OP, ident, bm_sb,
                  xres_in, out_ext, abrsbp, xrsp, outstp, tpool,
                  rs_c, c, rep):
    """Post-ReduceScatter: + bias_mean, transpose back, + residual, store."""
    rsb = abrsbp.tile([128, FSLICE // 128, NB], f32,
                      tag="abrsb", name=f"rsb{rep}_{c}")
    nc.gpsimd.dma_start(
        out=rsb, in_=rs_c.rearrange("(o p) n -> p o n", p=128))
    for j in range(FSLICE // 128):
        nc.scalar.activation(
            rsb[:, j, :], rsb[:, j, :], AF.Identity,
            bias=bm_sb[:, j:j + 1])
    for bp in range(NB // 256):
        r0 = c * NB + bp * 256
        xrs = xrsp.tile([128, 2, FSLICE], f32, tag="xrs")
        nc.sync.dma_start(
            out=xrs,
            in_=xres_in[r0:r0 + 256, :].rearrange("(o p) f -> p o f", p=128))
        obl = outstp.tile([128, 2, FSLICE // 128, 128], f32, tag="obl")
        for o in range(2):
            bsub = bp * 2 + o
            for j in range(FSLICE // 128):
                pt = tpool.tile([128, 128], f32, tag="tp")
                nc.tensor.transpose(
                    pt, rsb[:, j, bsub * 128:(bsub + 1) * 128], ident)
                nc.vector.tensor_tensor(
                    out=obl[:, o, j, :], in0=pt,
                    in1=xrs[:, o, j * 128:(j + 1) * 128], op=OP.add)
        nc.gpsimd.dma_start(
            out=out_ext[r0:r0 + 256, :].rearrange("(o p) f -> p o f", p=128),
            in_=obl.rearrange("p o a b -> p o (a b)"))



# revision 21
# speedup vs baseline: 2.3417x; 2.3417x over previous
"""GATv2 self-attention kernel for 8 Trainium2 NeuronCores.

Sharding: one attention head per core (8 heads / 8 cores). Each core computes
its head's attn-weighted projection as a partial sum over heads, the cores
ReduceScatter the partials over the feature axis, and each core finishes its
256-row feature slice (bias-mean + residual) in feature-major layout and
returns it transposed; the host re-transposes and concatenates.

Math per head h (reference):
  X = inputs.reshape(B*S, F); x0 = rows of X with s == 0
  Wh = leaky_relu(X @ W2h + broadcast_s(x0 @ W1h))      [B*S, F]
  e  = Wh @ att_w[h]; attn = softmax_s(e)
  out = sum_h (attn * Wh)/H + mean_h(bias) + X

All matmul operands are bf16, converted and pre-transposed on the host
(PSUM accumulation stays f32). X arrives already transposed so the PE does
no transposes at all. The broadcast x0@W1 term is produced directly in
natural (batch-major) layout by using x0^T as the matmul weights and W1 as
the data, then folded into the X@W2 PSUM groups through a 0/1 selector
matmul. att_w arrives replicated 128-wide so e lands broadcast across all
partitions and softmax runs at full DVE width with no broadcast step.

Schedule: 4 bs-chunks of 512. The x0@W1 prologue is interleaved into
chunk 0's fo loop (the selector for fo only needs the fc=fo//2 slice), and
chunk c's softmax/attn-mult/ReduceScatter tail overlaps chunk c+1's
matmuls. Finish phases for chunks 0/1 are emitted inside chunks 2/3.
"""

import sys
import numpy as np

sys.path.insert(0, "/opt/trn_rl_repo")

B, S, F, H = 256, 8, 2048, 8
BS = B * S            # 2048
NB = 512              # bs-chunk size (4 chunks)
NCHUNK = BS // NB     # 4
NFB = F // 128        # 16 feature blocks
NFC = 8               # W1 prologue column chunks (256 each)
FSLICE = F // H       # 256 output feature rows per core
NJ = FSLICE // 128    # 2
ALPHA = 0.3

_cache = {}
SIM_MODE = False  # replace collective with local DMA so TimelineSim can run


def _build(reps=1):
    import concourse.bacc as bacc
    import concourse.mybir as mybir
    import concourse.tile as tile
    import concourse.bass as bass

    f32 = mybir.dt.float32
    bf16 = mybir.dt.bfloat16
    AF = mybir.ActivationFunctionType
    OP = mybir.AluOpType

    nc = bacc.Bacc(num_devices=H)

    # host-prepared (bf16, pre-transposed) inputs
    w1_in = nc.declare_dram_parameter("w1t", [NFC, 128, NFB, 256], bf16,
                                      isOutput=False)
    w2_in = nc.declare_dram_parameter("w2t", [NFB, 128, NFB, 128], bf16,
                                      isOutput=False)
    xt_in = nc.declare_dram_parameter("xt", [128, NCHUNK, NFB, NB], bf16,
                                      isOutput=False)
    x0t_in = nc.declare_dram_parameter("x0t", [128, NFB, B], bf16,
                                       isOutput=False)
    sel_in = nc.declare_dram_parameter("selt", [128, NB], bf16, isOutput=False)
    aw_in = nc.declare_dram_parameter("attw", [128, NFB], f32, isOutput=False)
    xres_in = nc.declare_dram_parameter("xrest", [128, NCHUNK, NJ, NB], f32,
                                        isOutput=False)
    bm_in = nc.declare_dram_parameter("bm", [128, NJ], f32, isOutput=False)
    out_ext = nc.declare_dram_parameter("out", [FSLICE, BS], f32,
                                        isOutput=True)

    from contextlib import ExitStack
    with tile.TileContext(nc) as tc:
        with ExitStack() as ctx:
            consts = ctx.enter_context(tc.tile_pool(name="consts", bufs=1))
            xtp = ctx.enter_context(tc.tile_pool(name="xtp", bufs=1))
            whp = ctx.enter_context(tc.tile_pool(name="whp", bufs=2))
            w2p = ctx.enter_context(tc.tile_pool(name="w2p", bufs=4))
            w1p = ctx.enter_context(tc.tile_pool(name="w1p", bufs=2))
            x0tp = ctx.enter_context(tc.tile_pool(name="x0tp", bufs=1))
            x0cp = ctx.enter_context(tc.tile_pool(name="x0cp", bufs=1))
            attnp = ctx.enter_context(tc.tile_pool(name="attnp", bufs=2))
            esmp = ctx.enter_context(tc.tile_pool(name="esmp", bufs=2))
            smallp = ctx.enter_context(tc.tile_pool(name="smallp", bufs=2))
            xrsp = ctx.enter_context(tc.tile_pool(name="xrsp", bufs=2))
            rsbp = ctx.enter_context(tc.tile_pool(name="rsbp", bufs=2))
            outstp = ctx.enter_context(tc.tile_pool(name="outstp", bufs=2))
            ypool = ctx.enter_context(tc.tile_pool(name="ypool", bufs=3,
                                                   space="PSUM"))
            epool = ctx.enter_context(tc.tile_pool(name="epool", bufs=4,
                                                   space="PSUM"))
            dpool = ctx.enter_context(tc.tile_pool(name="dram", bufs=2,
                                                   space="DRAM"))

            # ---------------- constants (small; big loads are deferred) ----
            al_sb = consts.tile([128, 1], f32)
            nc.vector.memset(al_sb, ALPHA)
            bm_sb = consts.tile([128, NJ], f32)
            nc.sync.dma_start(out=bm_sb, in_=bm_in[:, :])
            sel_sb = consts.tile([128, NB], bf16)
            nc.sync.dma_start(out=sel_sb, in_=sel_in[:, :])
            aw_small = consts.tile([128, NFB], f32)
            nc.sync.dma_start(out=aw_small, in_=aw_in[:, :])
            # replicate att_w along a 128-wide free dim on-chip (cheap DVE op
            # instead of a 512 KiB DMA on the startup critical path)
            aw_sb = consts.tile([128, NFB, 128], bf16)
            nc.vector.tensor_copy(
                aw_sb, aw_small[:, :, None].to_broadcast((128, NFB, 128)))

            for _rep in range(reps):
                _run_body(nc, tc, mybir, bass, f32, bf16, AF, OP,
                          al_sb, bm_sb, sel_sb, aw_sb,
                          w1_in, w2_in, xt_in, x0t_in, xres_in, out_ext,
                          consts, xtp, whp, w2p, w1p, x0tp, x0cp,
                          attnp, esmp, smallp, xrsp, rsbp, outstp,
                          ypool, epool, dpool, _rep)

    nc.compile()
    return nc


def _run_body(nc, tc, mybir, bass, f32, bf16, AF, OP,
              al_sb, bm_sb, sel_sb, aw_sb,
              w1_in, w2_in, xt_in, x0t_in, xres_in, out_ext,
              consts, xtp, whp, w2p, w1p, x0tp, x0cp,
              attnp, esmp, smallp, xrsp, rsbp, outstp,
              ypool, epool, dpool, rep):
    AX = mybir.AxisListType

    # ---------------- input loads ----------------
    # x0t split in halves so the first prologue matmuls start sooner
    x0t = x0tp.tile([128, NFB, B], bf16, tag="x0t")
    nc.sync.dma_start(out=x0t[:, :NFB // 2], in_=x0t_in[:, :NFB // 2])
    nc.sync.dma_start(out=x0t[:, NFB // 2:], in_=x0t_in[:, NFB // 2:])

    # xt slab for chunk 0 now; slabs 1..3 are staggered into chunk c-1.
    # Issued from the ACT sequencer: the sync queue carries the W1/W2
    # streams and the Pool queue carries partial/RS/out traffic.
    xt = xtp.tile([128, NCHUNK, NFB, NB], bf16, tag="xt")
    nc.scalar.dma_start(out=xt[:, 0], in_=xt_in[:, 0])

    x0c = x0cp.tile([128, NJ, F], bf16, tag="x0c")

    def prologue_fc(fc):
        # x0c[b, f] = x0 @ W1 for a 256-col f chunk, natural layout:
        # weights = x0^T block [128 fi, 128 b], data = W1 [128 fi, 256 f]
        w1c = w1p.tile([128, NFB, 256], bf16, tag="w1c")
        nc.sync.dma_start(out=w1c, in_=w1_in[fc])
        for bh in range(2):
            ps = ypool.tile([128, 512], f32, tag="yp")
            for fi in range(NFB):
                nc.tensor.matmul(
                    ps[:, :256], x0t[:, fi, bh * 128:(bh + 1) * 128],
                    w1c[:, fi, :], start=(fi == 0), stop=(fi == NFB - 1))
            nc.scalar.activation(x0c[:, bh, fc * 256:(fc + 1) * 256],
                                 ps[:, :256], AF.Copy)

    rs_list = []
    wh_list = {}
    attn_list = {}

    def main_fo(c, fo, wh, e_ps, pending_e):
        w2b = w2p.tile([128, NFB, 128], bf16, tag="w2b")
        nc.sync.dma_start(out=w2b, in_=w2_in[fo])
        ps = ypool.tile([128, 512], f32, tag="yp")
        for fi in range(NFB):
            nc.tensor.matmul(
                ps, w2b[:, fi, :], xt[:, c, fi, :],
                start=(fi == 0), stop=False)
        # + broadcast_s(x0 @ W1): selector matmul closes the group.
        # chunk c covers 64 b-values living on partitions (c%2)*64..+64
        p0 = (c % 2) * 64
        nc.tensor.matmul(
            ps, x0c[p0:p0 + 64, c // 2, fo * 128:(fo + 1) * 128],
            sel_sb[p0:p0 + 64, :],
            start=False, stop=True)
        whs = wh[:, fo, :]
        nc.scalar.activation(whs, ps, AF.Prelu, alpha=al_sb[:, :])
        pending_e.append((fo, whs))
        while len(pending_e) > 1:
            efo, ewhs = pending_e.pop(0)
            nc.tensor.matmul(
                e_ps, aw_sb[:, efo, :], ewhs,
                start=(efo == 0), stop=(efo == NFB - 1))

    def chunk_tail(c, wh, e_ps, pending_e):
        for efo, ewhs in pending_e:
            nc.tensor.matmul(
                e_ps, aw_sb[:, efo, :], ewhs,
                start=(efo == 0), stop=(efo == NFB - 1))
        # softmax over s (groups of 8 along bs), scaled by 1/H. No max
        # subtraction: e is O(10) so exp cannot overflow f32. e is
        # replicated across all 128 partitions -> full-width DVE.
        NG = NB // S
        e_sb = esmp.tile([128, NB], f32, tag="esb", name=f"esb{rep}_{c}")
        nc.scalar.activation(e_sb, e_ps, AF.Exp)
        e3 = e_sb.rearrange("p (g s) -> p g s", s=S)
        sm = smallp.tile([128, NG], f32, tag="sm", name=f"sm{rep}_{c}")
        nc.vector.reduce_sum(out=sm, in_=e3, axis=AX.X)
        rc = smallp.tile([128, NG], f32, tag="rc", name=f"rc{rep}_{c}")
        nc.vector.reciprocal(rc, sm)
        nc.vector.tensor_scalar_mul(rc, rc, 1.0 / H)
        attn_sb = attnp.tile([128, NB], bf16, tag="attn", name=f"attn{rep}_{c}")
        a3 = attn_sb.rearrange("p (g s) -> p g s", s=S)
        nc.vector.tensor_tensor(
            out=a3, in0=e3, in1=rc[:, :, None].to_broadcast((128, NG, S)),
            op=OP.mult)

        # partial = attn/H * Wh in place (split across DVE and Pool), then
        # per-half DMA + ReduceScatter. Each half (features [0,1024) and
        # [1024,2048)) is reduced-scattered separately so the second half's
        # collective overlaps the first's.
        partial_c = dpool.tile([F, NB], bf16, tag="partial")
        pview = partial_c.rearrange("(o p) n -> p o n", p=128)
        rs_halves = []
        for half in range(2):
            lo, hi = half * (NFB // 2), (half + 1) * (NFB // 2)
            for fo in range(lo, hi):
                whs = wh[:, fo, :]
                nc.vector.tensor_tensor(out=whs, in0=whs, in1=attn_sb,
                                        op=OP.mult)
            nc.gpsimd.dma_start(out=pview[:, lo:hi, :], in_=wh[:, lo:hi, :])
            rs_h = dpool.tile([128, NB], bf16, tag=f"rs{half}",
                              name=f"rs{rep}_{c}_{half}")
            if SIM_MODE:
                nc.gpsimd.dma_start(
                    out=rs_h[:, :],
                    in_=partial_c[half * F // 2:half * F // 2 + 128, :])
            else:
                nc.gpsimd.collective_compute(
                    "ReduceScatter", OP.add,
                    replica_groups=[list(range(H))],
                    ins=[partial_c[half * F // 2:(half + 1) * F // 2, :]],
                    outs=[rs_h[:, :]])
            rs_halves.append(rs_h)
        rs_list.append(rs_halves)

    def finish(c):
        # post-ReduceScatter: + bias_mean (per-partition), + residual, store.
        # j=0 holds feature block 128h, j=1 holds block 1024+128h (the host
        # reassembles). rs halves are [128, NB] so they map to partitions
        # directly with no rearrange.
        rsb = rsbp.tile([128, NJ, NB], bf16, tag="rsb", name=f"rsb{rep}_{c}")
        xrs = xrsp.tile([128, NJ, NB], f32, tag="xrs", name=f"xrs{rep}_{c}")
        nc.sync.dma_start(out=xrs, in_=xres_in[:, c])
        obl = outstp.tile([128, NJ, NB], f32, tag="obl")
        oview = out_ext.rearrange("(o p) n -> p o n", p=128)
        for j in range(NJ):  # per-half so j=0 finish overlaps the j=1 RS
            nc.gpsimd.dma_start(out=rsb[:, j, :], in_=rs_list[c][j][:, :])
            nc.scalar.activation(obl[:, j, :], rsb[:, j, :],
                                 AF.Identity, bias=bm_sb[:, j:j + 1])
            nc.vector.tensor_tensor(out=obl[:, j, :], in0=obl[:, j, :],
                                    in1=xrs[:, j, :], op=OP.add)
            nc.gpsimd.dma_start(
                out=oview[:, j, c * NB:(c + 1) * NB], in_=obl[:, j, :])

    # ---------------- emission schedule ----------------
    prologue_fc(0)
    for c in range(NCHUNK):
        wh = whp.tile([128, NFB, NB], bf16, tag="wh", name=f"wh{rep}_{c}")
        e_ps = epool.tile([128, NB], f32, tag="ep", name=f"eps{rep}_{c}")
        pending_e = []
        for fo in range(NFB):
            if c == 0 and fo % 2 == 0 and fo // 2 + 1 < NFC:
                prologue_fc(fo // 2 + 1)
            if fo == 2 and c + 1 < NCHUNK:
                nc.scalar.dma_start(out=xt[:, c + 1], in_=xt_in[:, c + 1])
            if c == 2 and fo == 4:
                finish(0)
            if c == 3 and fo == 4:
                finish(1)
            main_fo(c, fo, wh, e_ps, pending_e)
        chunk_tail(c, wh, e_ps, pending_e)
    finish(2)
    finish(3)


def _get_nc():
    if "nc" not in _cache:
        _cache["nc"] = _build()
    return _cache["nc"]


def _make_in_maps(inputs_dict):
    import ml_dtypes
    bf16 = ml_dtypes.bfloat16

    x = np.asarray(inputs_dict["inputs"], dtype=np.float32).reshape(BS, F)
    W = np.asarray(inputs_dict["W"], dtype=np.float32)
    att_w = np.asarray(inputs_dict["att_w"], dtype=np.float32)
    bias = np.asarray(inputs_dict["bias"], dtype=np.float32)

    xT = np.ascontiguousarray(x.T)                      # [F, BS] f32
    x0 = x.reshape(B, S, F)[:, 0, :]                    # [B, F]

    # xt: [128 kp, NCHUNK, NFB fi, NB], chunk-major so slab DMAs are contiguous
    xt = np.ascontiguousarray(
        xT.reshape(NFB, 128, NCHUNK, NB).transpose(1, 2, 0, 3)).astype(bf16)
    # x0t: [128 kp, NFB fi, B]
    x0t = np.ascontiguousarray(
        x0.T.reshape(NFB, 128, B).transpose(1, 0, 2)).astype(bf16)
    # selt: [128, NB], sel[k, n] = (n // S == k % 64)
    eye = np.repeat(np.eye(64, dtype=np.float32), S, axis=1)   # [64, 512]
    sel = np.ascontiguousarray(np.tile(eye, (2, 1))).astype(bf16)

    bm_full = bias.mean(axis=0)                         # [F]

    def tile_w1(w):  # [F, F] -> [NFC fc, 128 kp, NFB fi, 256 n]
        return np.ascontiguousarray(
            w.reshape(NFB, 128, NFC, 256).transpose(2, 1, 0, 3)).astype(bf16)

    def tile_w2(w):  # [F, F] -> [NFB fo, 128 kp, NFB fi, 128 n]
        return np.ascontiguousarray(
            w.reshape(NFB, 128, NFB, 128).transpose(2, 1, 0, 3)).astype(bf16)

    in_maps = []
    for i in range(H):
        aw = np.ascontiguousarray(att_w[i].reshape(NFB, 128).T)  # [kp, fo]
        # core i owns feature blocks [128i, 128i+128) and [1024+128i, +128)
        # (one 128-row slice from each ReduceScatter half)
        fblk = [slice(128 * i, 128 * i + 128),
                slice(F // 2 + 128 * i, F // 2 + 128 * i + 128)]
        # xrest: [128 p, NCHUNK, NJ j, NB]
        xrest = np.stack([xT[fb].reshape(128, NCHUNK, NB) for fb in fblk],
                         axis=2)
        bm = np.stack([bm_full[fb] for fb in fblk], axis=1)  # [128, NJ]
        in_maps.append({
            "w1t": tile_w1(W[i, :F, :]),
            "w2t": tile_w2(W[i, F:, :]),
            "xt": xt,
            "x0t": x0t,
            "selt": sel,
            "attw": np.ascontiguousarray(aw),
            "xrest": np.ascontiguousarray(xrest),
            "bm": np.ascontiguousarray(bm),
        })
    return in_maps


def kernel(inputs, W, att_w, bias):
    from concourse.bass_utils import run_bass_kernel_spmd

    nc = _get_nc()
    in_maps = _make_in_maps(
        {"inputs": inputs, "W": W, "att_w": att_w, "bias": bias})
    res = run_bass_kernel_spmd(nc, in_maps, list(range(H)))
    _cache["last_result"] = res

    out_T = np.empty((F, BS), dtype=np.float32)
    for i in range(H):
        o = res.results[i]["out"]  # [FSLICE, BS]: rows j*128+p
        out_T[128 * i:128 * i + 128] = o[:128]
        out_T[F // 2 + 128 * i:F // 2 + 128 * i + 128] = o[128:]
    return np.ascontiguousarray(out_T.T).reshape(B, S, F)


# revision 27
# speedup vs baseline: 2.7033x; 1.1544x over previous
"""GATv2 self-attention kernel for 8 Trainium2 NeuronCores.

Sharding: one attention head per core (8 heads / 8 cores). Each core computes
its head's attn-weighted projection as a partial sum over heads, the cores
ReduceScatter the partials over the feature axis, and each core finishes its
256-row feature slice (bias-mean + residual) in feature-major layout and
returns it transposed; the host re-transposes and concatenates.

Math per head h (reference):
  X = inputs.reshape(B*S, F); x0 = rows of X with s == 0
  Wh = leaky_relu(X @ W2h + broadcast_s(x0 @ W1h))      [B*S, F]
  e  = Wh @ att_w[h]; attn = softmax_s(e)
  out = sum_h (attn * Wh)/H + mean_h(bias) + X

All matmul operands are bf16, converted and pre-transposed on the host
(PSUM accumulation stays f32). X arrives already transposed so the PE does
no transposes at all. The broadcast x0@W1 term is produced directly in
natural (batch-major) layout by using x0^T as the matmul weights and W1 as
the data, then folded into the X@W2 PSUM groups through a 0/1 selector
matmul. att_w arrives replicated 128-wide so e lands broadcast across all
partitions and softmax runs at full DVE width with no broadcast step.

Schedule: 4 bs-chunks of 512. The x0@W1 prologue is interleaved into
chunk 0's fo loop (the selector for fo only needs the fc=fo//2 slice), and
chunk c's softmax/attn-mult/ReduceScatter tail overlaps chunk c+1's
matmuls. Finish phases for chunks 0/1 are emitted inside chunks 2/3.
"""

import sys
import numpy as np

sys.path.insert(0, "/opt/trn_rl_repo")

B, S, F, H = 256, 8, 2048, 8
BS = B * S            # 2048
NB = 512              # bs-chunk size (4 chunks)
NCHUNK = BS // NB     # 4
NFB = F // 128        # 16 feature blocks
NFC = 8               # W1 prologue column chunks (256 each)
FSLICE = F // H       # 256 output feature rows per core
NJ = FSLICE // 128    # 2
ALPHA = 0.3

_cache = {}
SIM_MODE = False  # replace collective with local DMA so TimelineSim can run


def _build(reps=1):
    import concourse.bacc as bacc
    import concourse.mybir as mybir
    import concourse.tile as tile
    import concourse.bass as bass

    f32 = mybir.dt.float32
    bf16 = mybir.dt.bfloat16
    AF = mybir.ActivationFunctionType
    OP = mybir.AluOpType

    nc = bacc.Bacc(num_devices=H)

    # host-prepared (bf16, pre-transposed) inputs
    w1_in = nc.declare_dram_parameter("w1t", [NFC, 128, NFB, 256], bf16,
                                      isOutput=False)
    w2_in = nc.declare_dram_parameter("w2t", [NFB, 128, NFB, 128], bf16,
                                      isOutput=False)
    xt_in = nc.declare_dram_parameter("xt", [128, NCHUNK, NFB, NB], bf16,
                                      isOutput=False)
    x0t_in = nc.declare_dram_parameter("x0t", [128, NFB, B], bf16,
                                       isOutput=False)
    sel_in = nc.declare_dram_parameter("selt", [128, NB], bf16, isOutput=False)
    aw_in = nc.declare_dram_parameter("attw", [128, NFB], f32, isOutput=False)
    xres_in = nc.declare_dram_parameter("xrest", [128, NCHUNK, NJ, NB], f32,
                                        isOutput=False)
    bm_in = nc.declare_dram_parameter("bm", [128, NJ], f32, isOutput=False)
    out_ext = nc.declare_dram_parameter("out", [FSLICE, BS], f32,
                                        isOutput=True)

    from contextlib import ExitStack
    with tile.TileContext(nc) as tc:
        with ExitStack() as ctx:
            consts = ctx.enter_context(tc.tile_pool(name="consts", bufs=1))
            xtp = ctx.enter_context(tc.tile_pool(name="xtp", bufs=1))
            whp = ctx.enter_context(tc.tile_pool(name="whp", bufs=2))
            w2p = ctx.enter_context(tc.tile_pool(name="w2p", bufs=4))
            w1p = ctx.enter_context(tc.tile_pool(name="w1p", bufs=2))
            x0tp = ctx.enter_context(tc.tile_pool(name="x0tp", bufs=1))
            x0cp = ctx.enter_context(tc.tile_pool(name="x0cp", bufs=1))
            attnp = ctx.enter_context(tc.tile_pool(name="attnp", bufs=2))
            esmp = ctx.enter_context(tc.tile_pool(name="esmp", bufs=2))
            smallp = ctx.enter_context(tc.tile_pool(name="smallp", bufs=2))
            xrsp = ctx.enter_context(tc.tile_pool(name="xrsp", bufs=2))
            rsbp = ctx.enter_context(tc.tile_pool(name="rsbp", bufs=2))
            outstp = ctx.enter_context(tc.tile_pool(name="outstp", bufs=2))
            ypool = ctx.enter_context(tc.tile_pool(name="ypool", bufs=3,
                                                   space="PSUM"))
            epool = ctx.enter_context(tc.tile_pool(name="epool", bufs=4,
                                                   space="PSUM"))
            dpool = ctx.enter_context(tc.tile_pool(name="dram", bufs=2,
                                                   space="DRAM"))

            # ---------------- constants (small; big loads are deferred) ----
            al_sb = consts.tile([128, 1], f32)
            nc.vector.memset(al_sb, ALPHA)
            bm_sb = consts.tile([128, NJ], f32)
            nc.sync.dma_start(out=bm_sb, in_=bm_in[:, :])
            sel_sb = consts.tile([128, NB], bf16)
            nc.sync.dma_start(out=sel_sb, in_=sel_in[:, :])
            aw_small = consts.tile([128, NFB], f32)
            nc.sync.dma_start(out=aw_small, in_=aw_in[:, :])
            # replicate att_w along a 128-wide free dim on-chip (cheap DVE op
            # instead of a 512 KiB DMA on the startup critical path)
            aw_sb = consts.tile([128, NFB, 128], bf16)
            nc.vector.tensor_copy(
                aw_sb, aw_small[:, :, None].to_broadcast((128, NFB, 128)))

            for _rep in range(reps):
                _run_body(nc, tc, mybir, bass, f32, bf16, AF, OP,
                          al_sb, bm_sb, sel_sb, aw_sb,
                          w1_in, w2_in, xt_in, x0t_in, xres_in, out_ext,
                          consts, xtp, whp, w2p, w1p, x0tp, x0cp,
                          attnp, esmp, smallp, xrsp, rsbp, outstp,
                          ypool, epool, dpool, _rep)

    nc.compile()
    return nc


def _run_body(nc, tc, mybir, bass, f32, bf16, AF, OP,
              al_sb, bm_sb, sel_sb, aw_sb,
              w1_in, w2_in, xt_in, x0t_in, xres_in, out_ext,
              consts, xtp, whp, w2p, w1p, x0tp, x0cp,
              attnp, esmp, smallp, xrsp, rsbp, outstp,
              ypool, epool, dpool, rep):
    AX = mybir.AxisListType

    # ---------------- input loads ----------------
    # x0t split in halves so the first prologue matmuls start sooner
    x0t = x0tp.tile([128, NFB, B], bf16, tag="x0t")
    nc.sync.dma_start(out=x0t[:, :NFB // 2], in_=x0t_in[:, :NFB // 2])
    nc.sync.dma_start(out=x0t[:, NFB // 2:], in_=x0t_in[:, NFB // 2:])

    # xt slab for chunk 0 now; slabs 1..3 are staggered into chunk c-1.
    # Issued from the ACT sequencer: the sync queue carries the W1/W2
    # streams and the Pool queue carries partial/RS/out traffic.
    xt = xtp.tile([128, NCHUNK, NFB, NB], bf16, tag="xt")
    nc.scalar.dma_start(out=xt[:, 0], in_=xt_in[:, 0])

    x0c = x0cp.tile([128, NJ, F], bf16, tag="x0c")

    def prologue_fc(fc):
        # x0c[b, f] = x0 @ W1 for a 256-col f chunk, natural layout:
        # weights = x0^T block [128 fi, 128 b], data = W1 [128 fi, 256 f]
        w1c = w1p.tile([128, NFB, 256], bf16, tag="w1c")
        nc.sync.dma_start(out=w1c, in_=w1_in[fc])
        for bh in range(2):
            ps = ypool.tile([128, 512], f32, tag="yp")
            for fi in range(NFB):
                nc.tensor.matmul(
                    ps[:, :256], x0t[:, fi, bh * 128:(bh + 1) * 128],
                    w1c[:, fi, :], start=(fi == 0), stop=(fi == NFB - 1))
            nc.scalar.activation(x0c[:, bh, fc * 256:(fc + 1) * 256],
                                 ps[:, :256], AF.Copy)

    rs_list = []
    wh_list = {}
    attn_list = {}

    def main_fo(c, fo, wh, e_ps, pending_e):
        w2b = w2p.tile([128, NFB, 128], bf16, tag="w2b")
        nc.sync.dma_start(out=w2b, in_=w2_in[fo])
        ps = ypool.tile([128, 512], f32, tag="yp")
        for fi in range(NFB):
            nc.tensor.matmul(
                ps, w2b[:, fi, :], xt[:, c, fi, :],
                start=(fi == 0), stop=False)
        # + broadcast_s(x0 @ W1): selector matmul closes the group.
        # chunk c covers 64 b-values living on partitions (c%2)*64..+64
        p0 = (c % 2) * 64
        nc.tensor.matmul(
            ps, x0c[p0:p0 + 64, c // 2, fo * 128:(fo + 1) * 128],
            sel_sb[p0:p0 + 64, :],
            start=False, stop=True)
        whs = wh[:, fo, :]
        nc.scalar.activation(whs, ps, AF.Prelu, alpha=al_sb[:, :])
        pending_e.append((fo, whs))
        while len(pending_e) > 1:
            efo, ewhs = pending_e.pop(0)
            nc.tensor.matmul(
                e_ps, aw_sb[:, efo, :], ewhs,
                start=(efo == 0), stop=(efo == NFB - 1))

    def chunk_tail(c, wh, e_ps, pending_e):
        for efo, ewhs in pending_e:
            nc.tensor.matmul(
                e_ps, aw_sb[:, efo, :], ewhs,
                start=(efo == 0), stop=(efo == NFB - 1))
        # softmax over s (groups of 8 along bs), scaled by 1/H. No max
        # subtraction: e is O(10) so exp cannot overflow f32. e is
        # replicated across all 128 partitions -> full-width DVE.
        NG = NB // S
        e_sb = esmp.tile([128, NB], f32, tag="esb", name=f"esb{rep}_{c}")
        nc.scalar.activation(e_sb, e_ps, AF.Exp)
        e3 = e_sb.rearrange("p (g s) -> p g s", s=S)
        sm = smallp.tile([128, NG], f32, tag="sm", name=f"sm{rep}_{c}")
        nc.vector.reduce_sum(out=sm, in_=e3, axis=AX.X)
        rc = smallp.tile([128, NG], f32, tag="rc", name=f"rc{rep}_{c}")
        nc.vector.reciprocal(rc, sm)
        nc.vector.tensor_scalar_mul(rc, rc, 1.0 / H)
        attn_sb = attnp.tile([128, NB], bf16, tag="attn", name=f"attn{rep}_{c}")
        a3 = attn_sb.rearrange("p (g s) -> p g s", s=S)
        nc.vector.tensor_tensor(
            out=a3, in0=e3, in1=rc[:, :, None].to_broadcast((128, NG, S)),
            op=OP.mult)

        # partial = attn/H * Wh in place, then 2 bulk DMAs + one per-chunk
        # ReduceScatter (collectives have a ~20us fixed cost, so fewer and
        # bigger is better).
        partial_c = dpool.tile([F, NB], bf16, tag="partial")
        pview = partial_c.rearrange("(o p) n -> p o n", p=128)
        for half in range(2):
            lo, hi = half * (NFB // 2), (half + 1) * (NFB // 2)
            for fo in range(lo, hi):
                whs = wh[:, fo, :]
                nc.vector.tensor_tensor(out=whs, in0=whs, in1=attn_sb,
                                        op=OP.mult)
            nc.gpsimd.dma_start(out=pview[:, lo:hi, :], in_=wh[:, lo:hi, :])
        rs_c = dpool.tile([FSLICE, NB], bf16, tag="rs", name=f"rs{rep}_{c}")
        if SIM_MODE:
            nc.gpsimd.dma_start(out=rs_c[:, :], in_=partial_c[:FSLICE, :])
        else:
            nc.gpsimd.collective_compute(
                "ReduceScatter", OP.add,
                replica_groups=[list(range(H))],
                ins=[partial_c[:, :]], outs=[rs_c[:, :]])
        rs_list.append(rs_c)

    def finish(c):
        # post-ReduceScatter: + bias_mean (per-partition), + residual, store.
        # Feature row within the slice is j*128 + p.
        rsb = rsbp.tile([128, NJ, NB], bf16, tag="rsb", name=f"rsb{rep}_{c}")
        nc.gpsimd.dma_start(
            out=rsb, in_=rs_list[c].rearrange("(o p) n -> p o n", p=128))
        xrs = xrsp.tile([128, NJ, NB], f32, tag="xrs", name=f"xrs{rep}_{c}")
        nc.sync.dma_start(out=xrs, in_=xres_in[:, c])
        obl = outstp.tile([128, NJ, NB], f32, tag="obl")
        oview = out_ext.rearrange("(o p) n -> p o n", p=128)
        for j in range(NJ):
            nc.scalar.activation(obl[:, j, :], rsb[:, j, :],
                                 AF.Identity, bias=bm_sb[:, j:j + 1])
            nc.vector.tensor_tensor(out=obl[:, j, :], in0=obl[:, j, :],
                                    in1=xrs[:, j, :], op=OP.add)
        nc.gpsimd.dma_start(
            out=oview[:, :, c * NB:(c + 1) * NB], in_=obl)

    # ---------------- emission schedule ----------------
    prologue_fc(0)
    for c in range(NCHUNK):
        wh = whp.tile([128, NFB, NB], bf16, tag="wh", name=f"wh{rep}_{c}")
        e_ps = epool.tile([128, NB], f32, tag="ep", name=f"eps{rep}_{c}")
        pending_e = []
        for fo in range(NFB):
            if c == 0 and fo % 2 == 0 and fo // 2 + 1 < NFC:
                prologue_fc(fo // 2 + 1)
            if fo == 2 and c + 1 < NCHUNK:
                nc.scalar.dma_start(out=xt[:, c + 1], in_=xt_in[:, c + 1])
            if c == 2 and fo == 4:
                finish(0)
            if c == 3 and fo == 4:
                finish(1)
            main_fo(c, fo, wh, e_ps, pending_e)
        chunk_tail(c, wh, e_ps, pending_e)
    finish(2)
    finish(3)


def _get_nc():
    if "nc" not in _cache:
        _cache["nc"] = _build()
    return _cache["nc"]


def _make_in_maps(inputs_dict):
    import ml_dtypes
    bf16 = ml_dtypes.bfloat16

    x = np.asarray(inputs_dict["inputs"], dtype=np.float32).reshape(BS, F)
    W = np.asarray(inputs_dict["W"], dtype=np.float32)
    att_w = np.asarray(inputs_dict["att_w"], dtype=np.float32)
    bias = np.asarray(inputs_dict["bias"], dtype=np.float32)

    xT = np.ascontiguousarray(x.T)                      # [F, BS] f32
    x0 = x.reshape(B, S, F)[:, 0, :]                    # [B, F]

    # xt: [128 kp, NCHUNK, NFB fi, NB], chunk-major so slab DMAs are contiguous
    xt = np.ascontiguousarray(
        xT.reshape(NFB, 128, NCHUNK, NB).transpose(1, 2, 0, 3)).astype(bf16)
    # x0t: [128 kp, NFB fi, B]
    x0t = np.ascontiguousarray(
        x0.T.reshape(NFB, 128, B).transpose(1, 0, 2)).astype(bf16)
    # selt: [128, NB], sel[k, n] = (n // S == k % 64)
    eye = np.repeat(np.eye(64, dtype=np.float32), S, axis=1)   # [64, 512]
    sel = np.ascontiguousarray(np.tile(eye, (2, 1))).astype(bf16)

    bm_full = bias.mean(axis=0)                         # [F]

    def tile_w1(w):  # [F, F] -> [NFC fc, 128 kp, NFB fi, 256 n]
        return np.ascontiguousarray(
            w.reshape(NFB, 128, NFC, 256).transpose(2, 1, 0, 3)).astype(bf16)

    def tile_w2(w):  # [F, F] -> [NFB fo, 128 kp, NFB fi, 128 n]
        return np.ascontiguousarray(
            w.reshape(NFB, 128, NFB, 128).transpose(2, 1, 0, 3)).astype(bf16)

    in_maps = []
    for i in range(H):
        aw = np.ascontiguousarray(att_w[i].reshape(NFB, 128).T)  # [kp, fo]
        # core i owns the contiguous feature slice [256i, 256i+256);
        # feature row within the slice is j*128 + p
        fsl = slice(FSLICE * i, FSLICE * (i + 1))
        # xrest: [128 p, NCHUNK, NJ j, NB]
        xrest = xT[fsl].reshape(NJ, 128, NCHUNK, NB).transpose(1, 2, 0, 3)
        bm = bm_full[fsl].reshape(NJ, 128).T  # [128, NJ]
        in_maps.append({
            "w1t": tile_w1(W[i, :F, :]),
            "w2t": tile_w2(W[i, F:, :]),
            "xt": xt,
            "x0t": x0t,
            "selt": sel,
            "attw": np.ascontiguousarray(aw),
            "xrest": np.ascontiguousarray(xrest),
            "bm": np.ascontiguousarray(bm),
        })
    return in_maps


def kernel(inputs, W, att_w, bias):
    from concourse.bass_utils import run_bass_kernel_spmd

    nc = _get_nc()
    in_maps = _make_in_maps(
        {"inputs": inputs, "W": W, "att_w": att_w, "bias": bias})
    res = run_bass_kernel_spmd(nc, in_maps, list(range(H)))
    _cache["last_result"] = res

    out_T = np.concatenate([res.results[i]["out"] for i in range(H)], axis=0)
    return np.ascontiguousarray(out_T.T).reshape(B, S, F)
